# revision 93
# baseline (speedup 1.0000x reference)
"""Trainium2 Bass kernel for DynamicABPINN (moe_routing, dense evaluation).

Model: 8 gated subnets (4 hidden tanh layers of width 64 each), Gaussian-window
softmax gating over subnets, periodic input embedding, hard-constraint output.

Strategy:
  - Pure data parallel over 8 NeuronCores: each core handles N/8 = 131072 points.
  - Per core, three phases:
      S: point-major [128, 1024] whole-core ops (sin/cos embedding, tanh(t),
         gating squared-distance logits, per-point max via free-dim reduce),
         staged to DRAM so the main loop can read feature-major rows.
      M: 128 tiles of F=1024 points, feature-major. Per tile: gating exp via a
         small PE matmul (polynomial-expanded logits minus per-point max) + ACT
         Exp; MLP with 2-subnets-per-128-partitions block-diagonal weights, one
         fused ACT Tanh(scale*psum + bias) per pair-layer; PE partition-sum for
         the softmax numerator/denominator.
      F: point-major finalization u = tanh(t) * numer/denom + x^2 cos(pi x).
  - All transcendentals on ACT at [128, >=1024] granularity; no single-lane ops
    in hot loops.
"""

import sys

for _p in ("/opt/trn_rl_repo", "/root/.axon_site/_ro/trn_rl_repo"):
    if _p not in sys.path:
        sys.path.insert(0, _p)

import numpy as np

import concourse.bass as bass
import concourse.bacc as bacc
import concourse.mybir as mybir
from concourse.tile import TileContext
from concourse.tile_rust import add_dep_helper
from concourse.bass_utils import run_bass_kernel_spmd


def _strict_barrier(tc, nc):
    """strict_bb_all_engine_barrier, but anchored on a DRAIN instruction:
    walrus caps queue instructions at one embedded sem wait, except DRAIN
    (the end-of-context drain legally carries the full fan-in)."""
    curr_bb = nc.cur_bb
    assert curr_bb is not None
    prev_insts = list(curr_bb.bb.instructions)
    barrier_instruction = nc.sync.drain()
    tc.barrier_instruction_and_bb = (barrier_instruction.ins, curr_bb)
    if (
        tc.no_sync_barrier_and_bb is not None
        and tc.no_sync_barrier_and_bb[1] == curr_bb
    ):
        tc.no_sync_barrier_and_bb = None
    for instruction in prev_insts:
        add_dep_helper(
            barrier_instruction.ins,
            instruction,
            sync=bass.sync_unless_reorderable_target(
                instruction, instruction.is_executable()
            ),
            reason="strict_bb_all_engine_barrier: backward edge",
        )

F32 = mybir.dt.float32
AF = mybir.ActivationFunctionType
OP = mybir.AluOpType

N = 1048576
NCORES = 8
NC_PTS = N // NCORES          # 131072 points per core
P = 128                       # partitions
NJ = NC_PTS // P              # 1024 point-major columns
NT = P                        # 128 feature-major tiles of F=1024 points
F = NJ                        # 1024 points per tile
CH = 512                      # matmul moving-operand chunk (fp32 max, 1 psum bank)
K = 8                         # subnets
H = 64                        # hidden width
NPAIR = 4                     # subnet pairs packed into 128 partitions
PI = float(np.pi)

# column offsets inside the packed constant tensor
_COL_SIZES = (
    ("bsc", 16), ("ssc", 16), ("selB", 2), ("selO", 1),
    ("g5m", K), ("ebias", 1), ("cxn", K), ("ctn", K), ("gxv", K), ("gtv", K),
    ("trig", 2),
)
COL = {}
_off = 0
for _name, _sz in _COL_SIZES:
    COL[_name] = _off
    _off += _sz
CPACK_W = _off

# float32r weight pack (separate tensor: dtype differs from cpack)
_WCOL_SIZES = (
    ("w1", NPAIR * P), ("w2", NPAIR * P), ("w3", NPAIR * P), ("w4", NPAIR * P),
    ("w5", NPAIR * K),
)
WCOL = {}
_off = 0
for _name, _sz in _WCOL_SIZES:
    WCOL[_name] = _off
    _off += _sz
WPACK_W = _off

_CACHE = {}


def _build_program(debug=False):
    # Bacc (not plain Bass): its compile() runs generate_event_semaphores,
    # which splits multi-wait instructions into legal EventSemaphore chains.
    nc = bacc.Bacc()

    # I/O (per core)
    x_in = nc.declare_dram_parameter("x_in", [P, NJ], F32, isOutput=False)
    t_in = nc.declare_dram_parameter("t_in", [P, NJ], F32, isOutput=False)
    # All derived parameters packed into one tensor -> one DMA -> one
    # semaphore lane for every consumer ("too many sync waits" otherwise).
    cpack = nc.declare_dram_parameter("cpack", [P, CPACK_W], F32, isOutput=False)
    wpack = nc.declare_dram_parameter(
        "wpack", [P, WPACK_W], mybir.dt.float32r, isOutput=False
    )
    u_out = nc.declare_dram_parameter("u_out", [P, NJ], F32, isOutput=True)
    if debug:
        dbg_u0 = nc.declare_dram_parameter("dbg_u0", [P, NJ], F32, isOutput=True)
        dbg_th = nc.declare_dram_parameter("dbg_th", [P, NJ], F32, isOutput=True)
        dbg_mn = nc.declare_dram_parameter("dbg_mn", [P, NJ], F32, isOutput=True)
        dbg_si = nc.declare_dram_parameter(
            "dbg_si", [NT, 6, F], F32, isOutput=True
        )
        dbg_gf = nc.declare_dram_parameter(
            "dbg_gf", [NT, 5, F], F32, isOutput=True
        )
        dbg_cb = nc.declare_dram_parameter(
            "dbg_cb", [NT, 2, F], F32, isOutput=True
        )
        dbg_e = nc.declare_dram_parameter("dbg_e", [K, F], F32, isOutput=True)
        dbg_eo = nc.declare_dram_parameter("dbg_eo", [K, F], F32, isOutput=True)

    # Internal DRAM staging: the point-major -> feature-major "transpose".
    # stage_inp[c] = [cos, sin, t, cos, sin, t] rows for tile c (MLP input,
    # duplicated for the 2-subnet block-diagonal pairing).
    # stage_gf[c] = [x^2, x, t^2, t, mneg] rows for tile c (gating features).
    stage_inp = nc.dram_tensor("stage_inp", [NT, 6, F], mybir.dt.float32r)
    stage_gf = nc.dram_tensor("stage_gf", [NT, 5, F], F32)
    # per-tile [numer; D] rows, point-major on reload
    comb_dram = nc.dram_tensor("comb_dram", [NT, 2, F], F32)

    with TileContext(nc) as tc:
        with (
            tc.tile_pool(name="const", bufs=1) as cpool,
            tc.tile_pool(name="pm", bufs=1) as pm,
            tc.tile_pool(name="work", bufs=2) as wk,
            tc.tile_pool(name="hpool", bufs=12) as hp,
            tc.tile_pool(name="small", bufs=2) as sm,
            tc.tile_pool(name="epool", bufs=3) as ep,
            tc.tile_pool(name="psum", bufs=1, space="PSUM") as pp,
        ):
            # ---- constants to SBUF: one packed tile, sliced per use ----
            cpk = cpool.tile([P, CPACK_W], F32, tag="cpk")
            nc.sync.dma_start(out=cpk[:], in_=cpack[:])
            wpk = cpool.tile([P, WPACK_W], mybir.dt.float32r, tag="wpk")
            nc.sync.dma_start(out=wpk[:], in_=wpack[:])
            w1s = wpk[0:6, WCOL["w1"]:WCOL["w1"] + NPAIR * P]
            w2s = wpk[:, WCOL["w2"]:WCOL["w2"] + NPAIR * P]
            w3s = wpk[:, WCOL["w3"]:WCOL["w3"] + NPAIR * P]
            w4s = wpk[:, WCOL["w4"]:WCOL["w4"] + NPAIR * P]
            w5s = wpk[:, WCOL["w5"]:WCOL["w5"] + NPAIR * K]
            bscs = cpk[:, COL["bsc"]:COL["bsc"] + 16]
            sscs = cpk[:, COL["ssc"]:COL["ssc"] + 16]
            selBs = cpk[0:K, COL["selB"]:COL["selB"] + 2]
            selOs = cpk[0:K, COL["selO"]:COL["selO"] + 1]
            g5s = cpk[0:5, COL["g5m"]:COL["g5m"] + K]
            ebs = cpk[0:K, COL["ebias"]:COL["ebias"] + 1]
            cxns = cpk[:, COL["cxn"]:COL["cxn"] + K]
            ctns = cpk[:, COL["ctn"]:COL["ctn"] + K]
            gxvs = cpk[:, COL["gxv"]:COL["gxv"] + K]
            gtvs = cpk[:, COL["gtv"]:COL["gtv"] + K]
            trgc = cpk[:, COL["trig"]:COL["trig"] + 2]

            # Engine clock warmup: HW queue instructions carry at most ONE
            # embedded sem wait, so each engine first observes the cpack DMA
            # lane via a dummy op; later ops then wait only on their single
            # fresh dependency.
            wdum = sm.tile([1, 4], F32, tag="wdum")
            nc.scalar.activation(wdum[0:1, 0:1], cpk[0:1, 0:1], AF.Copy)

            # ---- Phase S: point-major prep ----
            # (scoped pool: everything except tanht/u0 dies once staged)
            tanht = pm.tile([P, NJ], F32, tag="tanht")
            u0 = pm.tile([P, NJ], F32, tag="u0")
            with tc.tile_pool(name="sphase", bufs=1) as sp:
                x_pm = sp.tile([P, NJ], F32, tag="x_pm")
                t_pm = sp.tile([P, NJ], F32, tag="t_pm")
                nc.sync.dma_start(out=x_pm[:], in_=x_in[:])
                nc.sync.dma_start(out=t_pm[:], in_=t_in[:])

                cosx = sp.tile([P, NJ], F32, tag="cosx")
                # fp32r copies for the MLP input path (ACT output rounds)
                F32R = mybir.dt.float32r
                cosr = sp.tile([P, NJ], F32R, tag="cosr")
                sinr = sp.tile([P, NJ], F32R, tag="sinr")
                t_r = sp.tile([P, NJ], F32R, tag="t_r")
                x2 = sp.tile([P, NJ], F32, tag="x2")
                t2 = sp.tile([P, NJ], F32, tag="t2")
                mneg = sp.tile([P, NJ], F32, tag="mneg")
                lg_all = sp.tile([P, NJ * (K // 2)], F32, tag="lg_all")

                # ACT Sin has no range reduction: reduce args to [-1, 1]
                # periods via fp32 magic-number rounding, r = z - 2*round(z/2).
                MAGIC = float(1.5 * 2 ** 23)
                scr1 = sp.tile([P, NJ], F32, tag="scr1")
                scr2 = sp.tile([P, NJ], F32, tag="scr2")
                # sin(pi x): r = x - 2 round(x/2)
                nc.vector.tensor_scalar(
                    out=scr1[:], in0=x_pm[:], scalar1=0.5, scalar2=MAGIC,
                    op0=OP.mult, op1=OP.add,
                )
                nc.vector.tensor_scalar(
                    out=scr1[:], in0=scr1[:], scalar1=MAGIC, scalar2=-2.0,
                    op0=OP.subtract, op1=OP.mult,
                )
                nc.vector.tensor_tensor(
                    out=scr1[:], in0=x_pm[:], in1=scr1[:], op=OP.add
                )
                nc.scalar.activation(
                    sinr[:], scr1[:], AF.Sin, bias=trgc[:, 1:2], scale=PI
                )
                # cos(pi x) = sin(pi (x + 1/2)): same reduction on y = x + 0.5
                # (+0.25 must precede the magic add: ULP at 1.5*2^23 is 1.0)
                nc.vector.tensor_scalar(
                    out=scr2[:], in0=x_pm[:], scalar1=0.5, scalar2=0.25,
                    op0=OP.mult, op1=OP.add,
                )
                nc.vector.tensor_scalar(
                    out=scr2[:], in0=scr2[:], scalar1=MAGIC, scalar2=MAGIC,
                    op0=OP.add, op1=OP.subtract,
                )
                nc.vector.tensor_scalar(
                    out=scr2[:], in0=scr2[:], scalar1=-2.0, scalar2=0.5,
                    op0=OP.mult, op1=OP.add,
                )
                nc.vector.tensor_tensor(
                    out=scr2[:], in0=x_pm[:], in1=scr2[:], op=OP.add
                )
                nc.scalar.activation(
                    cosx[:], scr2[:], AF.Sin, bias=trgc[:, 1:2], scale=PI
                )
                nc.scalar.activation(
                    cosr[:], scr2[:], AF.Sin, bias=trgc[:, 1:2], scale=PI
                )
                nc.scalar.activation(t_r[:], t_pm[:], AF.Copy)
                nc.scalar.activation(tanht[:], t_pm[:], AF.Tanh, bias=trgc[:, 1:2])
                nc.vector.tensor_tensor(
                    out=x2[:], in0=x_pm[:], in1=x_pm[:], op=OP.mult
                )
                nc.vector.tensor_tensor(
                    out=t2[:], in0=t_pm[:], in1=t_pm[:], op=OP.mult
                )
                # pin order: trig ACT ops before the Square loop, so later
                # consumers see their ticks as long-observed
                tc.no_sync_barrier()

                # gating: ssum_i = gx_i (x-cx_i)^2 + gt_i (t-ct_i)^2, point-major,
                # interleaved [P, NJ, K] so the per-point min is a free-dim
                # reduce. All-DVE so slot reuse costs only single self-waits.
                lg_v = lg_all[:].rearrange("p (j k) -> p j k", k=K // 2)
                for half in range(2):
                    for ii in range(K // 2):
                        i = half * (K // 2) + ii
                        nc.vector.tensor_scalar_add(
                            scr1[:], x_pm[:], cxns[:, i:i + 1]
                        )
                        nc.vector.tensor_tensor(
                            out=scr2[:], in0=scr1[:], in1=scr1[:], op=OP.mult
                        )
                        nc.vector.tensor_scalar_mul(
                            lg_v[:, :, ii], scr2[:], gxvs[:, i:i + 1]
                        )
                        nc.vector.tensor_scalar_add(
                            scr1[:], t_pm[:], ctns[:, i:i + 1]
                        )
                        nc.vector.tensor_tensor(
                            out=scr2[:], in0=scr1[:], in1=scr1[:], op=OP.mult
                        )
                        nc.vector.tensor_scalar_mul(
                            scr1[:], scr2[:], gtvs[:, i:i + 1]
                        )
                        nc.vector.tensor_tensor(
                            out=lg_v[:, :, ii], in0=lg_v[:, :, ii], in1=scr1[:],
                            op=OP.add,
                        )
                    dst = mneg if half == 0 else scr2
                    nc.vector.tensor_reduce(
                        out=dst[:], in_=lg_v, axis=mybir.AxisListType.X,
                        op=OP.min,
                    )
                nc.vector.tensor_tensor(
                    out=mneg[:], in0=mneg[:], in1=scr2[:], op=OP.min
                )
                tc.no_sync_barrier()
                # after the gating loop so its ACT dep (cosx) is long observed
                nc.vector.tensor_tensor(
                    out=u0[:], in0=x2[:], in1=cosx[:], op=OP.mult
                )

                # all-engine barrier: the stage DMAs below then carry no
                # compute waits (same-sequencer ordering after the barrier)
                _strict_barrier(tc, nc)
                for dst, r, src in (
                    (stage_gf, 1, x_pm), (stage_inp, 2, t_r),
                    (stage_inp, 5, t_r), (stage_gf, 3, t_pm),
                    (stage_inp, 0, cosr), (stage_inp, 3, cosr),
                    (stage_inp, 1, sinr), (stage_inp, 4, sinr),
                    (stage_gf, 0, x2), (stage_gf, 2, t2), (stage_gf, 4, mneg),
                ):
                    nc.sync.dma_start(out=dst[:, r, :], in_=src[:])

                if debug:
                    nc.sync.dma_start(out=dbg_u0[:], in_=u0[:])
                    nc.sync.dma_start(out=dbg_th[:], in_=tanht[:])
                    nc.sync.dma_start(out=dbg_mn[:], in_=mneg[:])
                    nc.sync.dma_start(
                        out=dbg_si[:], in_=stage_inp[:].bitcast(F32)
                    )
                    nc.sync.dma_start(out=dbg_gf[:], in_=stage_gf[:])

            # ---- Phase M: feature-major tile loop ----
            with (
                tc.tile_pool(name="mp_go", bufs=2, space="PSUM") as pgo,
                tc.tile_pool(name="mp_L", bufs=2, space="PSUM") as pL,
            ):
                # barrier: phase M starts with every engine having observed
                # all of phase S (each engine pays one sync wait, absorbed by
                # the warm ops below)
                _strict_barrier(tc, nc)
                wps = pgo.tile([K, F], F32, tag="go")
                nc.tensor.matmul(
                    out=wps[0:1, 0:2], lhsT=cpk[0:1, 0:1], rhs=cpk[0:1, 0:2],
                    start=True, stop=True,
                )
                wdum2 = sm.tile([1, 4], F32, tag="wdum")
                nc.scalar.activation(wdum2[0:1, 0:1], cpk[0:1, 0:1], AF.Copy)
                nc.vector.tensor_copy(out=wdum2[0:1, 1:2], in_=cpk[0:1, 0:1])

                F32R = mybir.dt.float32r
                wslice = (w1s, w2s, w3s, w4s)

                def emit_gate(c):
                    """Loads + gating exp for tile c."""
                    inp6 = wk.tile([6, F], F32R, tag="inp6", name=f"i6_{c}")
                    gf5 = wk.tile([5, F], F32, tag="gf5", name=f"g5_{c}")
                    nc.sync.dma_start(out=inp6[:], in_=stage_inp[c, :, :])
                    nc.sync.dma_start(out=gf5[:], in_=stage_gf[c, :, :])
                    lg_ps = pgo.tile([K, F], F32, tag="go", name=f"lg{c}")
                    for k in range(F // CH):
                        nc.tensor.matmul(
                            out=lg_ps[:, bass.ts(k, CH)], lhsT=g5s[:],
                            rhs=gf5[:, bass.ts(k, CH)], start=True, stop=True,
                        )
                    e_sb = ep.tile([K, F], F32, tag="e_sb", name=f"e{c}")
                    nc.scalar.activation(e_sb[0:1, 0:1], cpk[0:1, 0:1], AF.Copy)
                    nc.scalar.activation(
                        e_sb[:], lg_ps[:], AF.Exp, bias=ebs[0:K, 0:1]
                    )
                    return e_sb, inp6

                def emit_l1(c, inp6):
                    hs = []
                    for p in range(NPAIR):
                        lp = pL.tile([P, F], F32, tag="L", name=f"l1_{c}_{p}")
                        for k in range(F // CH):
                            nc.tensor.matmul(
                                out=lp[:, bass.ts(k, CH)],
                                lhsT=w1s[0:6, bass.ts(p, P)],
                                rhs=inp6[0:6, bass.ts(k, CH)],
                                start=True, stop=True,
                            )
                        hnew = hp.tile([P, F], F32R, tag="h", name=f"h1_{c}_{p}")
                        nc.scalar.activation(
                            hnew[:], lp[:], AF.Tanh,
                            bias=bscs[:, p:p + 1], scale=sscs[:, p:p + 1],
                        )
                        hs.append(hnew)
                    return hs

                def emit_layers(c, hs):
                    """Layers 2-4, head, and softmax combine for tile c."""
                    for l in range(1, 4):
                        wl = wslice[l]
                        hn = []
                        for p in range(NPAIR):
                            lp = pL.tile([P, F], F32, tag="L", name=f"l{l}_{c}_{p}")
                            for k in range(F // CH):
                                nc.tensor.matmul(
                                    out=lp[:, bass.ts(k, CH)],
                                    lhsT=wl[:, bass.ts(p, P)],
                                    rhs=hs[p][:, bass.ts(k, CH)],
                                    start=True, stop=True,
                                )
                            hnew = hp.tile(
                                [P, F], F32R, tag="h", name=f"h{l}_{c}_{p}"
                            )
                            nc.scalar.activation(
                                hnew[:], lp[:], AF.Tanh,
                                bias=bscs[:, 4 * l + p:4 * l + p + 1],
                                scale=sscs[:, 4 * l + p:4 * l + p + 1],
                            )
                            hn.append(hnew)
                        hs = hn
                    o_ps = pgo.tile([K, F], F32, tag="go", name=f"o{c}")
                    for p in range(NPAIR):
                        for k in range(F // CH):
                            nc.tensor.matmul(
                                out=o_ps[:, bass.ts(k, CH)],
                                lhsT=w5s[:, bass.ts(p, K)],
                                rhs=hs[p][:, bass.ts(k, CH)],
                                start=(p == 0), stop=(p == NPAIR - 1),
                            )
                    return o_ps

                def emit_tail(c, e_sb, o_ps):
                    # eo = e * o; selector matmuls place numer (row 0) and
                    # D (row 1); DVE drains [2, F] psum -> SBUF -> DRAM.
                    dtch = sm.tile([1, 4], F32, tag="dtch")
                    nc.vector.tensor_copy(out=dtch[0:1, 0:1], in_=e_sb[0:1, 0:1])
                    eo = sm.tile([K, F], F32, tag="eo", name=f"eo{c}")
                    nc.vector.tensor_tensor(
                        out=eo[:], in0=o_ps[:], in1=e_sb[:], op=OP.mult
                    )
                    place = pgo.tile([2, F], F32, tag="go", name=f"pl{c}")
                    for k in range(F // CH):
                        nc.tensor.matmul(
                            out=place[:, bass.ts(k, CH)], lhsT=selBs[:],
                            rhs=e_sb[:, bass.ts(k, CH)], start=True, stop=False,
                        )
                        nc.tensor.matmul(
                            out=place[0:1, bass.ts(k, CH)], lhsT=selOs[:],
                            rhs=eo[:, bass.ts(k, CH)], start=False, stop=True,
                        )
                    cdsb = sm.tile([2, F], F32, tag="cdsb", name=f"cd{c}")
                    nc.vector.memset(cdsb[0:1, 0:1], 0)
                    nc.vector.tensor_copy(out=cdsb[:], in_=place[:])
                    nc.sync.dma_start(out=comb_dram[c, :, :], in_=cdsb[:])
                    if debug and c == 0:
                        nc.sync.dma_start(out=dbg_e[:], in_=e_sb[:])
                        nc.sync.dma_start(out=dbg_eo[:], in_=eo[:])

                # software pipeline (2 tiles deep): per iteration c emit
                # L1(c+1), layers(c), gate(c+2), tail(c) - so ACT rolls from
                # tile-c tanhs into exp(c+2)/L1-tanh(c+1) while PE runs the
                # tile-c tail, and the gating matmul of c+2 is not stuck
                # behind place(c) in PE program order.
                e_cur, i_cur = emit_gate(0)
                h_cur = emit_l1(0, i_cur)
                gates = {0: (e_cur, i_cur)}
                gates[1] = emit_gate(1)
                for c in range(NT):
                    if c + 1 < NT:
                        h_next = emit_l1(c + 1, gates[c + 1][1])
                    o_ps = emit_layers(c, h_cur)
                    if c + 2 < NT:
                        gates[c + 2] = emit_gate(c + 2)
                    emit_tail(c, gates.pop(c)[0], o_ps)
                    if c + 1 < NT:
                        h_cur = h_next

            # ---- Phase F: DMA re-layout to full point-major, then finalize ----
            _strict_barrier(tc, nc)
            wdum3 = sm.tile([1, 4], F32, tag="wdum")
            nc.vector.tensor_copy(out=wdum3[0:1, 0:1], in_=cpk[0:1, 0:1])
            dND = pm.tile([P, 2 * NJ], F32, tag="dND")
            nc.sync.dma_start(
                out=dND[:], in_=comb_dram[:].rearrange("p r j -> p (r j)")
            )
            dN = dND[:, 0:NJ]
            dD = dND[:, NJ:2 * NJ]
            dinv = pm.tile([P, NJ], F32, tag="dinv")
            res = pm.tile([P, NJ], F32, tag="res")
            nc.vector.reciprocal(dinv[:], dD)
            nc.vector.tensor_tensor(out=res[:], in0=dN, in1=dinv[:], op=OP.mult)
            nc.vector.tensor_tensor(out=res[:], in0=res[:], in1=tanht[:], op=OP.mult)
            nc.vector.tensor_tensor(out=res[:], in0=res[:], in1=u0[:], op=OP.add)
            nc.sync.dma_start(out=u_out[:], in_=res[:])
            if debug:
                nc.sync.dma_start(out=dbg_cb[:], in_=comb_dram[:])

    nc.compile()
    return nc


def _prep_host(inputs):
    """Build the derived parameter arrays (tiny, replicated across cores)."""
    W1, b1 = inputs["W1"], inputs["b1"]      # [K,H,3], [K,H]
    W2, b2 = inputs["W2"], inputs["b2"]
    W3, b3 = inputs["W3"], inputs["b3"]
    W4, b4 = inputs["W4"], inputs["b4"]
    W5, b5 = inputs["W5"], inputs["b5"]      # [K,1,H], [K,1]
    scales = inputs["scales"]                # [K,4]
    centers = inputs["centers"]              # [K,2]
    log_gammas = inputs["log_gammas"]        # [K,2]

    f32 = np.float32
    w1l = np.zeros((6, NPAIR * P), f32)
    w2l = np.zeros((P, NPAIR * P), f32)
    w3l = np.zeros((P, NPAIR * P), f32)
    w4l = np.zeros((P, NPAIR * P), f32)
    w5l = np.zeros((P, NPAIR * K), f32)
    for p in range(NPAIR):
        a, b = 2 * p, 2 * p + 1
        w1l[0:3, p * P:p * P + H] = W1[a].T
        w1l[3:6, p * P + H:(p + 1) * P] = W1[b].T
        for wl, Wsrc in ((w2l, W2), (w3l, W3), (w4l, W4)):
            wl[0:H, p * P:p * P + H] = Wsrc[a].T
            wl[H:P, p * P + H:(p + 1) * P] = Wsrc[b].T
        w5l[0:H, p * K + a] = W5[a][0]
        w5l[H:P, p * K + b] = W5[b][0]

    bsc = np.zeros((P, 16), f32)
    ssc = np.zeros((P, 16), f32)
    blist = (b1, b2, b3, b4)
    for l in range(4):
        for p in range(NPAIR):
            a, b = 2 * p, 2 * p + 1
            col = 4 * l + p
            bsc[0:H, col] = scales[a, l] * blist[l][a]
            bsc[H:P, col] = scales[b, l] * blist[l][b]
            ssc[0:H, col] = scales[a, l]
            ssc[H:P, col] = scales[b, l]

    selB = np.zeros((K, 2), f32)
    selO = np.ones((K, 1), f32)
    selB[:, 0] = b5[:, 0]
    selB[:, 1] = 1.0

    gam = np.exp(log_gammas).astype(np.float64)
    cx, ct = centers[:, 0].astype(np.float64), centers[:, 1].astype(np.float64)
    gx, gt = gam[:, 0], gam[:, 1]
    g5m = np.zeros((5, K), f32)
    g5m[0] = -gx
    g5m[1] = 2.0 * gx * cx
    g5m[2] = -gt
    g5m[3] = 2.0 * gt * ct
    g5m[4] = 1.0
    ebias = (-(gx * cx * cx + gt * ct * ct)).astype(f32).reshape(K, 1)
    ones8 = np.ones((K, 1), f32)

    cxn = np.tile((-cx).astype(f32), (P, 1))
    ctn = np.tile((-ct).astype(f32), (P, 1))
    gxv = np.tile(gx.astype(f32), (P, 1))
    gtv = np.tile(gt.astype(f32), (P, 1))

    trigc = np.zeros((P, 2), f32)
    trigc[:, 0] = np.pi / 2

    cpack = np.zeros((P, CPACK_W), f32)
    wpack = np.zeros((P, WPACK_W), f32)

    def wput(name, arr):
        h, w = arr.shape
        wpack[0:h, WCOL[name]:WCOL[name] + w] = arr

    wput("w1", w1l)
    wput("w2", w2l)
    wput("w3", w3l)
    wput("w4", w4l)
    wput("w5", w5l)

    def put(name, arr):
        h, w = arr.shape
        cpack[0:h, COL[name]:COL[name] + w] = arr

    put("bsc", bsc)
    put("ssc", ssc)
    put("selB", selB)
    put("selO", selO)
    put("g5m", g5m)
    put("ebias", ebias)
    put("cxn", cxn)
    put("ctn", ctn)
    put("gxv", gxv)
    put("gtv", gtv)
    put("trig", trigc)
    return dict(cpack=cpack, wpack=wpack)


def kernel(**inputs):
    inputs = {k: np.asarray(v) for k, v in inputs.items()}
    x = inputs["x"].astype(np.float32).reshape(N)
    t = inputs["t"].astype(np.float32).reshape(N)

    if "nc" not in _CACHE:
        _CACHE["nc"] = _build_program()
    nc = _CACHE["nc"]

    params = _prep_host(inputs)
    in_maps = []
    for i in range(NCORES):
        sl = slice(i * NC_PTS, (i + 1) * NC_PTS)
        m = dict(params)
        m["x_in"] = np.ascontiguousarray(x[sl].reshape(P, NJ))
        m["t_in"] = np.ascontiguousarray(t[sl].reshape(P, NJ))
        in_maps.append(m)

    res = run_bass_kernel_spmd(nc, in_maps, list(range(NCORES)))
    out = np.empty((N,), np.float32)
    for i in range(NCORES):
        out[i * NC_PTS:(i + 1) * NC_PTS] = res.results[i]["u_out"].reshape(NC_PTS)
    return out.reshape(N, 1)


if __name__ == "__main__":
    rng = np.random.default_rng(0)
    print("smoke test: building program")
    _build_program()
    print("ok")


# revision 94
# speedup vs baseline: 1.0595x; 1.0595x over previous
"""Trainium2 Bass kernel for DynamicABPINN (moe_routing, dense evaluation).

Model: 8 gated subnets (4 hidden tanh layers of width 64 each), Gaussian-window
softmax gating over subnets, periodic input embedding, hard-constraint output.

Strategy:
  - Pure data parallel over 8 NeuronCores: each core handles N/8 = 131072 points.
  - Per core, three phases:
      S: point-major [128, 1024] whole-core ops (sin/cos embedding, tanh(t),
         gating squared-distance logits, per-point max via free-dim reduce),
         staged to DRAM so the main loop can read feature-major rows.
      M: 128 tiles of F=1024 points, feature-major. Per tile: gating exp via a
         small PE matmul (polynomial-expanded logits minus per-point max) + ACT
         Exp; MLP with 2-subnets-per-128-partitions block-diagonal weights, one
         fused ACT Tanh(scale*psum + bias) per pair-layer; PE partition-sum for
         the softmax numerator/denominator.
      F: point-major finalization u = tanh(t) * numer/denom + x^2 cos(pi x).
  - All transcendentals on ACT at [128, >=1024] granularity; no single-lane ops
    in hot loops.
"""

import sys

for _p in ("/opt/trn_rl_repo", "/root/.axon_site/_ro/trn_rl_repo"):
    if _p not in sys.path:
        sys.path.insert(0, _p)

import numpy as np

import concourse.bass as bass
import concourse.bacc as bacc
import concourse.mybir as mybir
from concourse.tile import TileContext
from concourse.tile_rust import add_dep_helper
from concourse.bass_utils import run_bass_kernel_spmd


def _strict_barrier(tc, nc):
    """strict_bb_all_engine_barrier, but anchored on a DRAIN instruction:
    walrus caps queue instructions at one embedded sem wait, except DRAIN
    (the end-of-context drain legally carries the full fan-in)."""
    curr_bb = nc.cur_bb
    assert curr_bb is not None
    prev_insts = list(curr_bb.bb.instructions)
    barrier_instruction = nc.sync.drain()
    tc.barrier_instruction_and_bb = (barrier_instruction.ins, curr_bb)
    if (
        tc.no_sync_barrier_and_bb is not None
        and tc.no_sync_barrier_and_bb[1] == curr_bb
    ):
        tc.no_sync_barrier_and_bb = None
    for instruction in prev_insts:
        add_dep_helper(
            barrier_instruction.ins,
            instruction,
            sync=bass.sync_unless_reorderable_target(
                instruction, instruction.is_executable()
            ),
            reason="strict_bb_all_engine_barrier: backward edge",
        )

F32 = mybir.dt.float32
AF = mybir.ActivationFunctionType
OP = mybir.AluOpType

N = 1048576
NCORES = 8
NC_PTS = N // NCORES          # 131072 points per core
P = 128                       # partitions
NJ = NC_PTS // P              # 1024 point-major columns
NT = P                        # 128 feature-major tiles of F=1024 points
F = NJ                        # 1024 points per tile
CH = 512                      # matmul moving-operand chunk (fp32 max, 1 psum bank)
K = 8                         # subnets
H = 64                        # hidden width
NPAIR = 4                     # subnet pairs packed into 128 partitions
PI = float(np.pi)

# column offsets inside the packed constant tensor
_COL_SIZES = (
    ("bsc", 16), ("ssc", 16), ("selB", 2), ("selO", 1),
    ("g5m", K), ("ebias", 1), ("cxn", K), ("ctn", K), ("gxv", K), ("gtv", K),
    ("trig", 2),
)
COL = {}
_off = 0
for _name, _sz in _COL_SIZES:
    COL[_name] = _off
    _off += _sz
CPACK_W = _off

# float32r weight pack (separate tensor: dtype differs from cpack)
_WCOL_SIZES = (
    ("w1", NPAIR * P), ("w2", NPAIR * P), ("w3", NPAIR * P), ("w4", NPAIR * P),
    ("w5", NPAIR * K),
)
WCOL = {}
_off = 0
for _name, _sz in _WCOL_SIZES:
    WCOL[_name] = _off
    _off += _sz
WPACK_W = _off

_CACHE = {}


def _build_program(debug=False):
    # Bacc (not plain Bass): its compile() runs generate_event_semaphores,
    # which splits multi-wait instructions into legal EventSemaphore chains.
    nc = bacc.Bacc()

    # I/O (per core)
    x_in = nc.declare_dram_parameter("x_in", [P, NJ], F32, isOutput=False)
    t_in = nc.declare_dram_parameter("t_in", [P, NJ], F32, isOutput=False)
    # All derived parameters packed into one tensor -> one DMA -> one
    # semaphore lane for every consumer ("too many sync waits" otherwise).
    cpack = nc.declare_dram_parameter("cpack", [P, CPACK_W], F32, isOutput=False)
    wpack = nc.declare_dram_parameter(
        "wpack", [P, WPACK_W], mybir.dt.float32r, isOutput=False
    )
    u_out = nc.declare_dram_parameter("u_out", [P, NJ], F32, isOutput=True)
    if debug:
        dbg_u0 = nc.declare_dram_parameter("dbg_u0", [P, NJ], F32, isOutput=True)
        dbg_th = nc.declare_dram_parameter("dbg_th", [P, NJ], F32, isOutput=True)
        dbg_mn = nc.declare_dram_parameter("dbg_mn", [P, NJ], F32, isOutput=True)
        dbg_si = nc.declare_dram_parameter(
            "dbg_si", [NT, 6, F], F32, isOutput=True
        )
        dbg_gf = nc.declare_dram_parameter(
            "dbg_gf", [NT, 5, F], F32, isOutput=True
        )
        dbg_cb = nc.declare_dram_parameter(
            "dbg_cb", [NT, 2, F], F32, isOutput=True
        )
        dbg_e = nc.declare_dram_parameter("dbg_e", [K, F], F32, isOutput=True)
        dbg_eo = nc.declare_dram_parameter("dbg_eo", [K, F], F32, isOutput=True)

    # Internal DRAM staging: the point-major -> feature-major "transpose".
    # stage_inp[c] = [cos, sin, t, cos, sin, t] rows for tile c (MLP input,
    # duplicated for the 2-subnet block-diagonal pairing).
    # stage_gf[c] = [x^2, x, t^2, t, mneg] rows for tile c (gating features).
    stage_inp = nc.dram_tensor("stage_inp", [NT, 6, F], mybir.dt.float32r)
    stage_gf = nc.dram_tensor("stage_gf", [NT, 5, F], F32)
    # per-tile [numer; D] rows, point-major on reload
    comb_dram = nc.dram_tensor("comb_dram", [NT, 2, F], F32)

    with TileContext(nc) as tc:
        with (
            tc.tile_pool(name="const", bufs=1) as cpool,
            tc.tile_pool(name="pm", bufs=1) as pm,
            tc.tile_pool(name="work", bufs=2) as wk,
            tc.tile_pool(name="hpool", bufs=12) as hp,
            tc.tile_pool(name="small", bufs=2) as sm,
            tc.tile_pool(name="epool", bufs=3) as ep,
            tc.tile_pool(name="psum", bufs=1, space="PSUM") as pp,
        ):
            # ---- constants to SBUF: one packed tile, sliced per use ----
            cpk = cpool.tile([P, CPACK_W], F32, tag="cpk")
            nc.sync.dma_start(out=cpk[:], in_=cpack[:])
            wpk = cpool.tile([P, WPACK_W], mybir.dt.float32r, tag="wpk")
            nc.sync.dma_start(out=wpk[:], in_=wpack[:])
            w1s = wpk[0:6, WCOL["w1"]:WCOL["w1"] + NPAIR * P]
            w2s = wpk[:, WCOL["w2"]:WCOL["w2"] + NPAIR * P]
            w3s = wpk[:, WCOL["w3"]:WCOL["w3"] + NPAIR * P]
            w4s = wpk[:, WCOL["w4"]:WCOL["w4"] + NPAIR * P]
            w5s = wpk[:, WCOL["w5"]:WCOL["w5"] + NPAIR * K]
            bscs = cpk[:, COL["bsc"]:COL["bsc"] + 16]
            sscs = cpk[:, COL["ssc"]:COL["ssc"] + 16]
            selBs = cpk[0:K, COL["selB"]:COL["selB"] + 2]
            selOs = cpk[0:K, COL["selO"]:COL["selO"] + 1]
            g5s = cpk[0:5, COL["g5m"]:COL["g5m"] + K]
            ebs = cpk[0:K, COL["ebias"]:COL["ebias"] + 1]
            cxns = cpk[:, COL["cxn"]:COL["cxn"] + K]
            ctns = cpk[:, COL["ctn"]:COL["ctn"] + K]
            gxvs = cpk[:, COL["gxv"]:COL["gxv"] + K]
            gtvs = cpk[:, COL["gtv"]:COL["gtv"] + K]
            trgc = cpk[:, COL["trig"]:COL["trig"] + 2]

            # Engine clock warmup: HW queue instructions carry at most ONE
            # embedded sem wait, so each engine first observes the cpack DMA
            # lane via a dummy op; later ops then wait only on their single
            # fresh dependency.
            wdum = sm.tile([1, 4], F32, tag="wdum")
            nc.scalar.activation(wdum[0:1, 0:1], cpk[0:1, 0:1], AF.Copy)

            # ---- Phase S: point-major prep ----
            # (scoped pool: everything except tanht/u0 dies once staged)
            tanht = pm.tile([P, NJ], F32, tag="tanht")
            u0 = pm.tile([P, NJ], F32, tag="u0")
            with tc.tile_pool(name="sphase", bufs=1) as sp:
                x_pm = sp.tile([P, NJ], F32, tag="x_pm")
                t_pm = sp.tile([P, NJ], F32, tag="t_pm")
                nc.sync.dma_start(out=x_pm[:], in_=x_in[:])
                nc.sync.dma_start(out=t_pm[:], in_=t_in[:])

                cosx = sp.tile([P, NJ], F32, tag="cosx")
                # fp32r copies for the MLP input path (ACT output rounds)
                F32R = mybir.dt.float32r
                cosr = sp.tile([P, NJ], F32R, tag="cosr")
                sinr = sp.tile([P, NJ], F32R, tag="sinr")
                t_r = sp.tile([P, NJ], F32R, tag="t_r")
                x2 = sp.tile([P, NJ], F32, tag="x2")
                t2 = sp.tile([P, NJ], F32, tag="t2")
                mneg = sp.tile([P, NJ], F32, tag="mneg")
                lg_all = sp.tile([P, NJ * (K // 2)], F32, tag="lg_all")

                # ACT Sin has no range reduction: reduce args to [-1, 1]
                # periods via fp32 magic-number rounding, r = z - 2*round(z/2).
                MAGIC = float(1.5 * 2 ** 23)
                scr1 = sp.tile([P, NJ], F32, tag="scr1")
                scr2 = sp.tile([P, NJ], F32, tag="scr2")
                # sin(pi x): r = x - 2 round(x/2)
                nc.vector.tensor_scalar(
                    out=scr1[:], in0=x_pm[:], scalar1=0.5, scalar2=MAGIC,
                    op0=OP.mult, op1=OP.add,
                )
                nc.vector.tensor_scalar(
                    out=scr1[:], in0=scr1[:], scalar1=MAGIC, scalar2=-2.0,
                    op0=OP.subtract, op1=OP.mult,
                )
                nc.vector.tensor_tensor(
                    out=scr1[:], in0=x_pm[:], in1=scr1[:], op=OP.add
                )
                nc.scalar.activation(
                    sinr[:], scr1[:], AF.Sin, bias=trgc[:, 1:2], scale=PI
                )
                # cos(pi x) = sin(pi (x + 1/2)): same reduction on y = x + 0.5
                # (+0.25 must precede the magic add: ULP at 1.5*2^23 is 1.0)
                nc.vector.tensor_scalar(
                    out=scr2[:], in0=x_pm[:], scalar1=0.5, scalar2=0.25,
                    op0=OP.mult, op1=OP.add,
                )
                nc.vector.tensor_scalar(
                    out=scr2[:], in0=scr2[:], scalar1=MAGIC, scalar2=MAGIC,
                    op0=OP.add, op1=OP.subtract,
                )
                nc.vector.tensor_scalar(
                    out=scr2[:], in0=scr2[:], scalar1=-2.0, scalar2=0.5,
                    op0=OP.mult, op1=OP.add,
                )
                nc.vector.tensor_tensor(
                    out=scr2[:], in0=x_pm[:], in1=scr2[:], op=OP.add
                )
                nc.scalar.activation(
                    cosx[:], scr2[:], AF.Sin, bias=trgc[:, 1:2], scale=PI
                )
                nc.scalar.activation(
                    cosr[:], scr2[:], AF.Sin, bias=trgc[:, 1:2], scale=PI
                )
                nc.scalar.activation(t_r[:], t_pm[:], AF.Copy)
                nc.scalar.activation(tanht[:], t_pm[:], AF.Tanh, bias=trgc[:, 1:2])
                nc.vector.tensor_tensor(
                    out=x2[:], in0=x_pm[:], in1=x_pm[:], op=OP.mult
                )
                nc.vector.tensor_tensor(
                    out=t2[:], in0=t_pm[:], in1=t_pm[:], op=OP.mult
                )
                # pin order: trig ACT ops before the Square loop, so later
                # consumers see their ticks as long-observed
                tc.no_sync_barrier()

                # gating: ssum_i = gx_i (x-cx_i)^2 + gt_i (t-ct_i)^2, point-major,
                # interleaved [P, NJ, K] so the per-point min is a free-dim
                # reduce. All-DVE so slot reuse costs only single self-waits.
                lg_v = lg_all[:].rearrange("p (j k) -> p j k", k=K // 2)
                for half in range(2):
                    for ii in range(K // 2):
                        i = half * (K // 2) + ii
                        nc.vector.tensor_scalar_add(
                            scr1[:], x_pm[:], cxns[:, i:i + 1]
                        )
                        nc.vector.tensor_tensor(
                            out=scr2[:], in0=scr1[:], in1=scr1[:], op=OP.mult
                        )
                        nc.vector.tensor_scalar_mul(
                            lg_v[:, :, ii], scr2[:], gxvs[:, i:i + 1]
                        )
                        nc.vector.tensor_scalar_add(
                            scr1[:], t_pm[:], ctns[:, i:i + 1]
                        )
                        nc.vector.tensor_tensor(
                            out=scr2[:], in0=scr1[:], in1=scr1[:], op=OP.mult
                        )
                        nc.vector.tensor_scalar_mul(
                            scr1[:], scr2[:], gtvs[:, i:i + 1]
                        )
                        nc.vector.tensor_tensor(
                            out=lg_v[:, :, ii], in0=lg_v[:, :, ii], in1=scr1[:],
                            op=OP.add,
                        )
                    dst = mneg if half == 0 else scr2
                    nc.vector.tensor_reduce(
                        out=dst[:], in_=lg_v, axis=mybir.AxisListType.X,
                        op=OP.min,
                    )
                nc.vector.tensor_tensor(
                    out=mneg[:], in0=mneg[:], in1=scr2[:], op=OP.min
                )
                tc.no_sync_barrier()
                # after the gating loop so its ACT dep (cosx) is long observed
                nc.vector.tensor_tensor(
                    out=u0[:], in0=x2[:], in1=cosx[:], op=OP.mult
                )

                # all-engine barrier: the stage DMAs below then carry no
                # compute waits (same-sequencer ordering after the barrier)
                _strict_barrier(tc, nc)
                for dst, r, src in (
                    (stage_gf, 1, x_pm), (stage_inp, 2, t_r),
                    (stage_inp, 5, t_r), (stage_gf, 3, t_pm),
                    (stage_inp, 0, cosr), (stage_inp, 3, cosr),
                    (stage_inp, 1, sinr), (stage_inp, 4, sinr),
                    (stage_gf, 0, x2), (stage_gf, 2, t2), (stage_gf, 4, mneg),
                ):
                    nc.sync.dma_start(out=dst[:, r, :], in_=src[:])

                if debug:
                    nc.sync.dma_start(out=dbg_u0[:], in_=u0[:])
                    nc.sync.dma_start(out=dbg_th[:], in_=tanht[:])
                    nc.sync.dma_start(out=dbg_mn[:], in_=mneg[:])
                    nc.sync.dma_start(
                        out=dbg_si[:], in_=stage_inp[:].bitcast(F32)
                    )
                    nc.sync.dma_start(out=dbg_gf[:], in_=stage_gf[:])

            # ---- Phase M: feature-major tile loop ----
            with (
                tc.tile_pool(name="mp_go", bufs=2, space="PSUM") as pgo,
                tc.tile_pool(name="mp_L", bufs=2, space="PSUM") as pL,
            ):
                # barrier: phase M starts with every engine having observed
                # all of phase S (each engine pays one sync wait, absorbed by
                # the warm ops below)
                _strict_barrier(tc, nc)
                wps = pgo.tile([K, F], F32, tag="go")
                nc.tensor.matmul(
                    out=wps[0:1, 0:2], lhsT=cpk[0:1, 0:1], rhs=cpk[0:1, 0:2],
                    start=True, stop=True,
                )
                wdum2 = sm.tile([1, 4], F32, tag="wdum")
                nc.scalar.activation(wdum2[0:1, 0:1], cpk[0:1, 0:1], AF.Copy)
                nc.vector.tensor_copy(out=wdum2[0:1, 1:2], in_=cpk[0:1, 0:1])

                F32R = mybir.dt.float32r
                wslice = (w1s, w2s, w3s, w4s)

                def emit_gate(c):
                    """Loads + gating exp for tile c."""
                    inp6 = wk.tile([6, F], F32R, tag="inp6", name=f"i6_{c}")
                    gf5 = wk.tile([5, F], F32, tag="gf5", name=f"g5_{c}")
                    nc.sync.dma_start(out=inp6[:], in_=stage_inp[c, :, :])
                    nc.sync.dma_start(out=gf5[:], in_=stage_gf[c, :, :])
                    lg_ps = pgo.tile([K, F], F32, tag="go", name=f"lg{c}")
                    for k in range(F // CH):
                        nc.tensor.matmul(
                            out=lg_ps[:, bass.ts(k, CH)], lhsT=g5s[:],
                            rhs=gf5[:, bass.ts(k, CH)], start=True, stop=True,
                        )
                    e_sb = ep.tile([K, F], F32, tag="e_sb", name=f"e{c}")
                    nc.scalar.activation(
                        e_sb[:], lg_ps[:], AF.Exp, bias=ebs[0:K, 0:1]
                    )
                    return e_sb, inp6

                def emit_l1(c, inp6):
                    hs = []
                    for p in range(NPAIR):
                        lp = pL.tile([P, F], F32, tag="L", name=f"l1_{c}_{p}")
                        for k in range(F // CH):
                            nc.tensor.matmul(
                                out=lp[:, bass.ts(k, CH)],
                                lhsT=w1s[0:6, bass.ts(p, P)],
                                rhs=inp6[0:6, bass.ts(k, CH)],
                                start=True, stop=True,
                            )
                        hnew = hp.tile([P, F], F32R, tag="h", name=f"h1_{c}_{p}")
                        nc.scalar.activation(
                            hnew[:], lp[:], AF.Tanh,
                            bias=bscs[:, p:p + 1], scale=sscs[:, p:p + 1],
                        )
                        hs.append(hnew)
                    return hs

                def emit_layers(c, hs):
                    """Layers 2-4, head, and softmax combine for tile c."""
                    for l in range(1, 4):
                        wl = wslice[l]
                        hn = []
                        for p in range(NPAIR):
                            lp = pL.tile([P, F], F32, tag="L", name=f"l{l}_{c}_{p}")
                            for k in range(F // CH):
                                nc.tensor.matmul(
                                    out=lp[:, bass.ts(k, CH)],
                                    lhsT=wl[:, bass.ts(p, P)],
                                    rhs=hs[p][:, bass.ts(k, CH)],
                                    start=True, stop=True,
                                )
                            hnew = hp.tile(
                                [P, F], F32R, tag="h", name=f"h{l}_{c}_{p}"
                            )
                            nc.scalar.activation(
                                hnew[:], lp[:], AF.Tanh,
                                bias=bscs[:, 4 * l + p:4 * l + p + 1],
                                scale=sscs[:, 4 * l + p:4 * l + p + 1],
                            )
                            hn.append(hnew)
                        hs = hn
                    o_ps = pgo.tile([K, F], F32, tag="go", name=f"o{c}")
                    for p in range(NPAIR):
                        for k in range(F // CH):
                            nc.tensor.matmul(
                                out=o_ps[:, bass.ts(k, CH)],
                                lhsT=w5s[:, bass.ts(p, K)],
                                rhs=hs[p][:, bass.ts(k, CH)],
                                start=(p == 0), stop=(p == NPAIR - 1),
                            )
                    return o_ps

                def emit_tail(c, e_sb, o_ps):
                    # eo = e * o; selector matmuls place numer (row 0) and
                    # D (row 1); DVE drains [2, F] psum -> SBUF -> DRAM.
                    eo = sm.tile([K, F], F32, tag="eo", name=f"eo{c}")
                    nc.vector.tensor_tensor(
                        out=eo[:], in0=o_ps[:], in1=e_sb[:], op=OP.mult
                    )
                    place = pgo.tile([2, F], F32, tag="go", name=f"pl{c}")
                    for k in range(F // CH):
                        nc.tensor.matmul(
                            out=place[:, bass.ts(k, CH)], lhsT=selBs[:],
                            rhs=e_sb[:, bass.ts(k, CH)], start=True, stop=False,
                        )
                        nc.tensor.matmul(
                            out=place[0:1, bass.ts(k, CH)], lhsT=selOs[:],
                            rhs=eo[:, bass.ts(k, CH)], start=False, stop=True,
                        )
                    cdsb = sm.tile([2, F], F32, tag="cdsb", name=f"cd{c}")
                    nc.vector.tensor_copy(out=cdsb[:], in_=place[:])
                    nc.sync.dma_start(out=comb_dram[c, :, :], in_=cdsb[:])
                    if debug and c == 0:
                        nc.sync.dma_start(out=dbg_e[:], in_=e_sb[:])
                        nc.sync.dma_start(out=dbg_eo[:], in_=eo[:])

                # software pipeline (2 tiles deep): per iteration c emit
                # L1(c+1), layers(c), gate(c+2), tail(c) - so ACT rolls from
                # tile-c tanhs into exp(c+2)/L1-tanh(c+1) while PE runs the
                # tile-c tail, and the gating matmul of c+2 is not stuck
                # behind place(c) in PE program order.
                e_cur, i_cur = emit_gate(0)
                h_cur = emit_l1(0, i_cur)
                gates = {0: (e_cur, i_cur)}
                gates[1] = emit_gate(1)
                for c in range(NT):
                    o_ps = emit_layers(c, h_cur)
                    if c + 2 < NT:
                        gates[c + 2] = emit_gate(c + 2)
                    if c + 1 < NT:
                        h_cur = emit_l1(c + 1, gates[c + 1][1])
                    emit_tail(c, gates.pop(c)[0], o_ps)

            # ---- Phase F: DMA re-layout to full point-major, then finalize ----
            _strict_barrier(tc, nc)
            wdum3 = sm.tile([1, 4], F32, tag="wdum")
            nc.vector.tensor_copy(out=wdum3[0:1, 0:1], in_=cpk[0:1, 0:1])
            dND = pm.tile([P, 2 * NJ], F32, tag="dND")
            nc.sync.dma_start(
                out=dND[:], in_=comb_dram[:].rearrange("p r j -> p (r j)")
            )
            dN = dND[:, 0:NJ]
            dD = dND[:, NJ:2 * NJ]
            dinv = pm.tile([P, NJ], F32, tag="dinv")
            res = pm.tile([P, NJ], F32, tag="res")
            nc.vector.reciprocal(dinv[:], dD)
            nc.vector.tensor_tensor(out=res[:], in0=dN, in1=dinv[:], op=OP.mult)
            nc.vector.tensor_tensor(out=res[:], in0=res[:], in1=tanht[:], op=OP.mult)
            nc.vector.tensor_tensor(out=res[:], in0=res[:], in1=u0[:], op=OP.add)
            nc.sync.dma_start(out=u_out[:], in_=res[:])
            if debug:
                nc.sync.dma_start(out=dbg_cb[:], in_=comb_dram[:])

    nc.compile()
    return nc


def _prep_host(inputs):
    """Build the derived parameter arrays (tiny, replicated across cores)."""
    W1, b1 = inputs["W1"], inputs["b1"]      # [K,H,3], [K,H]
    W2, b2 = inputs["W2"], inputs["b2"]
    W3, b3 = inputs["W3"], inputs["b3"]
    W4, b4 = inputs["W4"], inputs["b4"]
    W5, b5 = inputs["W5"], inputs["b5"]      # [K,1,H], [K,1]
    scales = inputs["scales"]                # [K,4]
    centers = inputs["centers"]              # [K,2]
    log_gammas = inputs["log_gammas"]        # [K,2]

    f32 = np.float32
    w1l = np.zeros((6, NPAIR * P), f32)
    w2l = np.zeros((P, NPAIR * P), f32)
    w3l = np.zeros((P, NPAIR * P), f32)
    w4l = np.zeros((P, NPAIR * P), f32)
    w5l = np.zeros((P, NPAIR * K), f32)
    for p in range(NPAIR):
        a, b = 2 * p, 2 * p + 1
        w1l[0:3, p * P:p * P + H] = W1[a].T
        w1l[3:6, p * P + H:(p + 1) * P] = W1[b].T
        for wl, Wsrc in ((w2l, W2), (w3l, W3), (w4l, W4)):
            wl[0:H, p * P:p * P + H] = Wsrc[a].T
            wl[H:P, p * P + H:(p + 1) * P] = Wsrc[b].T
        w5l[0:H, p * K + a] = W5[a][0]
        w5l[H:P, p * K + b] = W5[b][0]

    bsc = np.zeros((P, 16), f32)
    ssc = np.zeros((P, 16), f32)
    blist = (b1, b2, b3, b4)
    for l in range(4):
        for p in range(NPAIR):
            a, b = 2 * p, 2 * p + 1
            col = 4 * l + p
            bsc[0:H, col] = scales[a, l] * blist[l][a]
            bsc[H:P, col] = scales[b, l] * blist[l][b]
            ssc[0:H, col] = scales[a, l]
            ssc[H:P, col] = scales[b, l]

    selB = np.zeros((K, 2), f32)
    selO = np.ones((K, 1), f32)
    selB[:, 0] = b5[:, 0]
    selB[:, 1] = 1.0

    gam = np.exp(log_gammas).astype(np.float64)
    cx, ct = centers[:, 0].astype(np.float64), centers[:, 1].astype(np.float64)
    gx, gt = gam[:, 0], gam[:, 1]
    g5m = np.zeros((5, K), f32)
    g5m[0] = -gx
    g5m[1] = 2.0 * gx * cx
    g5m[2] = -gt
    g5m[3] = 2.0 * gt * ct
    g5m[4] = 1.0
    ebias = (-(gx * cx * cx + gt * ct * ct)).astype(f32).reshape(K, 1)
    ones8 = np.ones((K, 1), f32)

    cxn = np.tile((-cx).astype(f32), (P, 1))
    ctn = np.tile((-ct).astype(f32), (P, 1))
    gxv = np.tile(gx.astype(f32), (P, 1))
    gtv = np.tile(gt.astype(f32), (P, 1))

    trigc = np.zeros((P, 2), f32)
    trigc[:, 0] = np.pi / 2

    cpack = np.zeros((P, CPACK_W), f32)
    wpack = np.zeros((P, WPACK_W), f32)

    def wput(name, arr):
        h, w = arr.shape
        wpack[0:h, WCOL[name]:WCOL[name] + w] = arr

    wput("w1", w1l)
    wput("w2", w2l)
    wput("w3", w3l)
    wput("w4", w4l)
    wput("w5", w5l)

    def put(name, arr):
        h, w = arr.shape
        cpack[0:h, COL[name]:COL[name] + w] = arr

    put("bsc", bsc)
    put("ssc", ssc)
    put("selB", selB)
    put("selO", selO)
    put("g5m", g5m)
    put("ebias", ebias)
    put("cxn", cxn)
    put("ctn", ctn)
    put("gxv", gxv)
    put("gtv", gtv)
    put("trig", trigc)
    return dict(cpack=cpack, wpack=wpack)


def kernel(**inputs):
    inputs = {k: np.asarray(v) for k, v in inputs.items()}
    x = inputs["x"].astype(np.float32).reshape(N)
    t = inputs["t"].astype(np.float32).reshape(N)

    if "nc" not in _CACHE:
        _CACHE["nc"] = _build_program()
    nc = _CACHE["nc"]

    params = _prep_host(inputs)
    in_maps = []
    for i in range(NCORES):
        sl = slice(i * NC_PTS, (i + 1) * NC_PTS)
        m = dict(params)
        m["x_in"] = np.ascontiguousarray(x[sl].reshape(P, NJ))
        m["t_in"] = np.ascontiguousarray(t[sl].reshape(P, NJ))
        in_maps.append(m)

    res = run_bass_kernel_spmd(nc, in_maps, list(range(NCORES)))
    out = np.empty((N,), np.float32)
    for i in range(NCORES):
        out[i * NC_PTS:(i + 1) * NC_PTS] = res.results[i]["u_out"].reshape(NC_PTS)
    return out.reshape(N, 1)


if __name__ == "__main__":
    rng = np.random.default_rng(0)
    print("smoke test: building program")
    _build_program()
    print("ok")
